# revision 32
# baseline (speedup 1.0000x reference)
"""Trainium2 Bass kernel for nn_Bottleneck_7911329759669 (topk_masking bottleneck).

Self-contained: builds the Bass module on first call, runs SPMD on 8 NeuronCores
(data-parallel over batch, 8 samples per core), returns the full output.

v2 design (per core, 8 samples):
  - x loaded once per sample as exact fp32 [128, 2*3136] (one DMA).  Exact x
    feeds: spatial saliency (fp32 transposed matmuls), channel pooling
    (gpsimd reduce), and a 2x-mode DVE cast to a bf16 copy xb; then it dies.
  - spatial saliency TRANSPOSED: 56 tiny fp32 matmuls, stationary = strided
    x window [128, 112] (columns = pixels 28m+f), moving = mask_w column ->
    psum [112, 28] in u-layout sal[p, f] = sal(28p + f).  Exact fp32, so the
    top-k set matches the reference (order-stat spacing ~6e-4 >> fp32 noise).
  - conv1/ident/conv2/conv3/dilation/mask-broadcast matmuls in bf16
    (masks are 0/1 = exact; value-path rounding ~0.5% << 2e-2 tolerance).
  - channel top-32: gpsimd pooled sums -> fp32 fc matmul -> sigmoid ->
    exact pairwise-count trick.
  - spatial top-1568: 32-step bitwise bisection on sortable-u32 (exact tie
    semantics); one pair of samples per bisect; counts via ones-matmul.
  - 3x3 dilation: ones9-matmul over 9 shifted copies of the padded mask row
    (sh9 built by ONE 9-descriptor overlapping-stride DMA); dil = min(cnt,1)
    multiplied into t12 on DVE.
  - conv2 3x3 = 6 matmuls/chunk on the row-padded bf16 layout (stride 58):
    3 K=128 tap-pairs (dx=-1,0 via a +1-pre-shifted partition copy) + 3 K=64
    singles.
  - conv3 K=65 bf16 (bn3 scale folded into weights; bn3-bias*mask as row 64,
    where rhs65 is PADDED layout and row 64 is the mask row itself) + bf16
    eye matmul adding identity xb into the same PSUM; ReLU on ScalarE during
    eviction into bf16 ybuf; y stored bf16 (one DMA per m-half) and widened
    to fp32 on the host (<0.4% error, halves the y DMA).

mask_b is ignored: adding a constant to the saliency cannot change its top-k
mask, and the saliency itself is not part of the output.
"""
import os
import sys

for _p in ("/opt/trn_rl_repo",):
    if _p not in sys.path:
        sys.path.insert(0, _p)

import numpy as np

import concourse.bass as bass
import concourse.tile as tile
from concourse import bacc, mybir

F32 = mybir.dt.float32
BF16 = mybir.dt.bfloat16
U32 = mybir.dt.uint32
I32 = mybir.dt.int32
OP = mybir.AluOpType
AF = mybir.ActivationFunctionType
AX = mybir.AxisListType

B, CIN, H, W = 64, 256, 56, 56
WIDTH, COUT = 64, 256
N = H * W                      # 3136
K_SP, K_CH = 1568, 32
EPS = 1e-5
NCORES = 8
SPC = B // NCORES              # 8 samples per core

PW = W + 2                     # padded row stride
BASE = 64
NP = BASE + PW * H + BASE      # 3376
CH = 448                       # pixels per chunk (8 rows)
NCH = N // CH                  # 7
RPC = CH // W                  # 8 rows per chunk

UP, UF = 112, 28               # 112*28 == 3136
PAIR = 2

DEBUG = bool(int(os.environ.get("KDEBUG", "0")))


def _padded(t, p0, p1, chunk, off):
    """[p1-p0, 8, 56] view of padded tile t at pixel chunk `chunk` shifted by off."""
    start = BASE + PW * RPC * chunk + off
    return t[p0:p1, start:start + PW * RPC].rearrange("p (h w) -> p h w", h=RPC)[:, :, 0:W]


def _build_nc():
    nc = bacc.Bacc("TRN2", target_bir_lowering=False, debug=False)

    x_d = nc.dram_tensor("x", [SPC, CIN, N], F32, kind="ExternalInput").ap()
    c1w_d = nc.dram_tensor("conv1_w", [WIDTH, CIN], F32, kind="ExternalInput").ap()
    bn1 = {k: nc.dram_tensor(f"bn1_{k}", [WIDTH], F32, kind="ExternalInput").ap() for k in "gbmv"}
    c2w_d = nc.dram_tensor("conv2_w", [WIDTH, WIDTH, 3, 3], F32, kind="ExternalInput").ap()
    bn2 = {k: nc.dram_tensor(f"bn2_{k}", [WIDTH], F32, kind="ExternalInput").ap() for k in "gbmv"}
    c3w_d = nc.dram_tensor("conv3_w", [COUT, WIDTH], F32, kind="ExternalInput").ap()
    bn3 = {k: nc.dram_tensor(f"bn3_{k}", [COUT], F32, kind="ExternalInput").ap() for k in "gbmv"}
    fcw_d = nc.dram_tensor("fc_w", [WIDTH, CIN], F32, kind="ExternalInput").ap()
    fcb_d = nc.dram_tensor("fc_b", [WIDTH], F32, kind="ExternalInput").ap()
    mw_d = nc.dram_tensor("mask_w", [CIN], F32, kind="ExternalInput").ap()
    nc.dram_tensor("mask_b", [1], F32, kind="ExternalInput")  # unused (constant shift)
    y_d = nc.dram_tensor("y", [SPC, COUT, N], BF16, kind="ExternalOutput").ap()

    dbg = {}
    if DEBUG:
        dbg["sal"] = nc.dram_tensor("dbg_sal", [SPC, 64], F32, kind="ExternalOutput").ap()
        dbg["vec"] = nc.dram_tensor("dbg_vec", [SPC, 64], F32, kind="ExternalOutput").ap()
        dbg["sp"] = nc.dram_tensor("dbg_sp", [SPC, UP, UF], F32, kind="ExternalOutput").ap()
        dbg["u"] = nc.dram_tensor("dbg_u", [SPC, UP, UF], U32, kind="ExternalOutput").ap()
        dbg["mask"] = nc.dram_tensor("dbg_mask", [SPC, UP, UF], F32, kind="ExternalOutput").ap()
        dbg["t12"] = nc.dram_tensor("dbg_t12", [SPC, 128, NP], F32, kind="ExternalOutput").ap()

    eye128_d = nc.inline_tensor(np.eye(128, dtype=np.float32), "eye128").ap()
    onesum_d = nc.inline_tensor(np.ones((UP, 128), np.float32), "ones_sum").ap()
    ones1x64_d = nc.inline_tensor(np.ones((1, 64), np.float32), "ones1x64").ap()
    ones9_d = nc.inline_tensor(np.ones((9, 64), np.float32), "ones9x64").ap()

    from contextlib import ExitStack
    with tile.TileContext(nc) as tc, ExitStack() as ctx:
        _body(ctx, tc, nc, x_d, y_d, c1w_d, bn1, c2w_d, bn2, c3w_d, bn3,
              fcw_d, fcb_d, mw_d, eye128_d, onesum_d, ones1x64_d, ones9_d, dbg)
    nc.compile()
    return nc


def _body(ctx, tc, nc, x_d, y_d, c1w_d, bn1, c2w_d, bn2, c3w_d, bn3,
          fcw_d, fcb_d, mw_d, eye128_d, onesum_d, ones1x64_d, ones9_d, dbg):
    consts = ctx.enter_context(tc.tile_pool(name="consts", bufs=1))
    xpool = ctx.enter_context(tc.tile_pool(name="xp", bufs=2))
    xbpool = ctx.enter_context(tc.tile_pool(name="xbp", bufs=4 if DEBUG else 5))
    t12p = ctx.enter_context(tc.tile_pool(name="t12p", bufs=4))
    rhs65p = ctx.enter_context(tc.tile_pool(name="rhs65p", bufs=2))
    sh9p = ctx.enter_context(tc.tile_pool(name="sh9p", bufs=2))
    ybufp = ctx.enter_context(tc.tile_pool(name="ybufp", bufs=3))
    smallp = ctx.enter_context(tc.tile_pool(name="smalls", bufs=6))
    upool = ctx.enter_context(tc.tile_pool(name="utiles", bufs=10))
    outp = ctx.enter_context(tc.tile_pool(name="outs", bufs=4))
    ps_z1 = ctx.enter_context(tc.tile_pool(name="ps_z1", bufs=1, space="PSUM"))
    ps_z2 = ctx.enter_context(tc.tile_pool(name="ps_z2", bufs=2, space="PSUM"))
    ps_dm = ctx.enter_context(tc.tile_pool(name="ps_dm", bufs=2, space="PSUM"))
    ps_z3 = ctx.enter_context(tc.tile_pool(name="ps_z3", bufs=2, space="PSUM"))
    ps_sm = ctx.enter_context(tc.tile_pool(name="ps_sm", bufs=1, space="PSUM"))

    # ---------- constants ----------
    ident = consts.tile([128, 128], F32)
    nc.sync.dma_start(ident, eye128_d)
    eye_b = consts.tile([128, 128], BF16)
    nc.vector.tensor_copy(eye_b, ident)
    onesum = consts.tile([UP, 128], F32)
    nc.sync.dma_start(onesum, onesum_d)
    ones1x64 = consts.tile([1, 64], F32)
    nc.sync.dma_start(ones1x64, ones1x64_d)
    ones1b = consts.tile([1, 64], BF16)
    nc.vector.tensor_copy(ones1b, ones1x64)
    ones_at64 = consts.tile([65, 64], BF16)
    nc.vector.memset(ones_at64, 1.0)
    ones9f = smallp.tile([9, 64], F32, tag="wstage9", bufs=1)
    nc.sync.dma_start(ones9f, ones9_d)
    ones9b = consts.tile([9, 64], BF16)
    nc.vector.tensor_copy(ones9b, ones9f)

    # u32 bit-pattern constant columns (immediates >= 2^31 are unreliable)
    bits = consts.tile([UP, 33], U32)
    for k in range(32):
        nc.vector.memset(bits[:, k:k + 1], 1 << k)
    nc.vector.memset(bits[:, 32:33], 0x80000000)

    # conv1 lhsT bf16: two [128, 64] K-tiles
    w1b = []
    for k in range(2):
        tf = smallp.tile([128, 64], F32, tag="wstage", bufs=2)
        nc.sync.dma_start(tf, c1w_d.transpose([1, 0])[128 * k:128 * (k + 1), :])
        t = consts.tile([128, 64], BF16, name=f"w1b_{k}")
        nc.vector.tensor_copy(t, tf)
        w1b.append(t)

    # mask_w as two [128, 1] fp32 K-columns (moving operand of sal-T matmuls)
    mwcol = []
    for k in range(2):
        c = consts.tile([128, 1], F32, name=f"mw_{k}")
        nc.sync.dma_start(c, mw_d[128 * k:128 * (k + 1)].unsqueeze(1))
        mwcol.append(c)

    # fc lhsT: two [128, 64] fp32 K-tiles; fc_b as [64,1]
    fcw = []
    for k in range(2):
        t = consts.tile([128, 64], F32, name=f"fcw_{k}")
        nc.sync.dma_start(t, fcw_d.transpose([1, 0])[128 * k:128 * (k + 1), :])
        fcw.append(t)
    fcb_col = consts.tile([64, 1], F32)
    nc.sync.dma_start(fcb_col, fcb_d.unsqueeze(1))

    # conv2 taps bf16
    def tap_ap(dy, dx):
        return c2w_d[:, :, dy + 1, dx + 1].transpose([1, 0])

    w2pair, w2sing = [], []
    for dy in (-1, 0, 1):
        tf = smallp.tile([128, 64], F32, tag="wstage", bufs=2)
        nc.sync.dma_start(tf[0:64], tap_ap(dy, -1))
        nc.sync.dma_start(tf[64:128], tap_ap(dy, 0))
        t = consts.tile([128, 64], BF16, name=f"w2p_{dy + 1}")
        nc.vector.tensor_copy(t, tf)
        w2pair.append(t)
        sf = smallp.tile([64, 64], F32, tag="wstage2", bufs=2)
        nc.sync.dma_start(sf, tap_ap(dy, 1))
        s = consts.tile([64, 64], BF16, name=f"w2s_{dy + 1}")
        nc.vector.tensor_copy(s, sf)
        w2sing.append(s)

    eps64 = consts.tile([64, 1], F32)
    nc.vector.memset(eps64, EPS)
    eps2 = consts.tile([2, 1], F32)
    nc.vector.memset(eps2, EPS)

    # bn1 / bn2 scale+bias columns [64,1]
    def bn_prep64(bnd, nm):
        cols = {}
        for k in "gbmv":
            c = smallp.tile([64, 1], F32, name=f"{nm}_{k}", tag=f"{nm}_{k}", bufs=1)
            nc.sync.dma_start(c, bnd[k].unsqueeze(1))
            cols[k] = c
        sd = smallp.tile([64, 1], F32, name=f"{nm}_sd", tag=f"{nm}_sd", bufs=1)
        nc.scalar.activation(sd, cols["v"], AF.Sqrt, bias=eps64, scale=1.0)
        rs = smallp.tile([64, 1], F32, name=f"{nm}_rs", tag=f"{nm}_rs", bufs=1)
        nc.vector.reciprocal(rs, sd)
        s = consts.tile([64, 1], F32, name=f"{nm}_s")
        nc.vector.tensor_mul(s, cols["g"], rs)
        bp = consts.tile([64, 1], F32, name=f"{nm}_bp")
        nc.vector.tensor_mul(bp, cols["m"], s)
        nc.vector.tensor_sub(bp, cols["b"], bp)
        return s, bp

    s1c, b1c = bn_prep64(bn1, "bn1")
    s2c, b2c = bn_prep64(bn2, "bn2")

    # bn3 in [2,128] layout (c = 128*p + f), then conv3 lhsT [65, 256] bf16
    def load_2x128(d, nm):
        t = smallp.tile([2, 128], F32, name=nm, tag=nm, bufs=1)
        nc.sync.dma_start(t, d.rearrange("(p f) -> p f", p=2))
        return t

    g3 = load_2x128(bn3["g"], "g3")
    b3 = load_2x128(bn3["b"], "b3")
    m3 = load_2x128(bn3["m"], "m3")
    v3 = load_2x128(bn3["v"], "v3")
    sd3 = smallp.tile([2, 128], F32, tag="sd3", bufs=1)
    nc.scalar.activation(sd3, v3, AF.Sqrt, bias=eps2, scale=1.0)
    rs3 = smallp.tile([2, 128], F32, tag="rs3", bufs=1)
    nc.vector.reciprocal(rs3, sd3)
    s3 = smallp.tile([2, 128], F32, tag="s3", bufs=1)
    nc.vector.tensor_mul(s3, g3, rs3)
    b3p = smallp.tile([2, 128], F32, tag="b3p", bufs=1)
    nc.vector.tensor_mul(b3p, m3, s3)
    nc.vector.tensor_sub(b3p, b3, b3p)

    w3f = smallp.tile([65, 256], F32, tag="w3f", bufs=1)
    nc.sync.dma_start(w3f[0:64], c3w_d.transpose([1, 0]))
    s3row = smallp.tile([1, 256], F32, tag="s3row", bufs=1)
    nc.sync.dma_start(s3row, s3)          # [2,128] -> [1,256] partition-major
    nc.sync.dma_start(w3f[64:65], b3p)
    s3b = ps_sm.tile([64, 256], F32, tag="sm")
    nc.tensor.matmul(s3b, ones1x64, s3row, start=True, stop=True)
    w3 = consts.tile([65, 256], BF16)
    nc.vector.scalar_tensor_tensor(w3[0:64], s3b, 1.0, w3f[0:64],
                                   op0=OP.bypass, op1=OP.mult)
    nc.vector.tensor_copy(w3[64:65], w3f[64:65])

    class S:
        pass

    # ---------------- stage A (per sample) ----------------
    def stage_a(s):
        st = S()
        # one DMA: x[s] [256, 3136] -> [128, 2*3136] (channel c+128 at col N+j)
        xt = xpool.tile([128, 2 * N], F32, name=f"x_s{s}", tag="x")
        xsrc = bass.AP(x_d.tensor, s * CIN * N, [[N, 128], [128 * N, 2], [1, N]])
        nc.sync.dma_start(xt, xsrc)
        st.xt = xt

        # spatial saliency, transposed: salp[m, f] = sal(28m + f), exact fp32
        salp = ps_sm.tile([UP, UF], F32, tag="sm")
        for f in range(UF):
            for k in range(2):
                win = xt[:, k * N:(k + 1) * N].rearrange("p (m f) -> p m f", f=UF)
                lhsT = win[:, :, f:f + 1].rearrange("p m f -> p (m f)")
                nc.tensor.matmul(salp[:, f:f + 1], lhsT, mwcol[k],
                                 start=(k == 0), stop=(k == 1))

        # sortable-u32 transform: u = bits ^ (sign ? 0xFFFFFFFF : 0x80000000)
        st.u = upool.tile([UP, UF], U32, name=f"u_s{s}", tag="u")
        nc.vector.tensor_copy(st.u.bitcast(F32), salp)
        if DEBUG:
            nc.sync.dma_start(dbg["sp"][s], st.u.bitcast(F32))
        bb = upool.tile([UP, UF], U32, tag="bb")
        nc.vector.tensor_scalar(bb.bitcast(I32), st.u.bitcast(I32),
                                31, None, op0=OP.arith_shift_right)
        nc.vector.tensor_tensor(bb, bb, bits[:, 32:33].broadcast_to([UP, UF]),
                                op=OP.bitwise_or)
        nc.vector.tensor_tensor(st.u, st.u, bb, op=OP.bitwise_xor)
        if DEBUG:
            nc.sync.dma_start(dbg["u"][s], st.u)

        # bf16 cast fused with pooled sums (2x-mode DVE tensor_scalar + accum)
        xb = xbpool.tile([128, 2 * N], BF16, name=f"xb_s{s}", tag="xb")
        st.xb = xb
        pool0 = smallp.tile([128, 1], F32, tag="pool0")
        pool1 = smallp.tile([128, 1], F32, tag="pool1")
        nc.vector.tensor_scalar(xb[:, 0:N], xt[:, 0:N], 0.0, 0.0,
                                op0=OP.add, op1=OP.add, accum_out=pool0)
        nc.vector.tensor_scalar(xb[:, N:2 * N], xt[:, N:2 * N], 0.0, 0.0,
                                op0=OP.add, op1=OP.add, accum_out=pool1)
        fcps = ps_sm.tile([64, 1], F32, tag="sm")
        nc.tensor.matmul(fcps, fcw[0], pool0, start=True, stop=False)
        nc.tensor.matmul(fcps, fcw[1], pool1, start=False, stop=True)
        sal = smallp.tile([64, 1], F32, tag="sal")
        nc.scalar.activation(sal, fcps, AF.Sigmoid, bias=fcb_col, scale=1.0 / N)
        salT = ps_sm.tile([1, 64], F32, tag="sm")
        nc.tensor.transpose(salT, sal, ident[0:64, 0:64])
        salrow = smallp.tile([1, 64], F32, tag="salrow")
        nc.scalar.copy(salrow, salT)
        aps = ps_sm.tile([64, 64], F32, tag="sm")
        nc.tensor.matmul(aps, ones1x64, salrow, start=True, stop=True)
        scr = smallp.tile([64, 64], F32, tag="scr")
        cnt = smallp.tile([64, 1], F32, tag="cnt")
        # in1 must be SBUF: DVE has a single PSUM read port (in0=aps is PSUM)
        nc.vector.scalar_tensor_tensor(scr, aps, sal, sal.broadcast_to([64, 64]),
                                       op0=OP.is_gt, op1=OP.bypass, accum_out=cnt)
        vec = smallp.tile([64, 1], F32, tag="vec")
        nc.vector.tensor_scalar(vec, cnt, float(K_CH), None, op0=OP.is_lt)
        if DEBUG:
            nc.sync.dma_start(dbg["sal"][s], sal)
            nc.sync.dma_start(dbg["vec"][s], vec)
        st.s1v = smallp.tile([64, 1], F32, tag="s1v")
        nc.vector.tensor_mul(st.s1v, s1c, vec)
        st.b1v = smallp.tile([64, 1], F32, tag="b1v")
        nc.vector.tensor_mul(st.b1v, b1c, vec)
        st.s2v = smallp.tile([64, 1], F32, tag="s2v")
        nc.vector.tensor_mul(st.s2v, s2c, vec)
        st.b2v = smallp.tile([64, 1], F32, tag="b2v")
        nc.vector.tensor_mul(st.b2v, b2c, vec)

        # conv1 bf16 + bn1-relu eviction into padded bf16 t12 (pads stay 0)
        t12 = t12p.tile([128, NP], BF16, name=f"t12_s{s}", tag="t12")
        if s < 4:
            nc.gpsimd.memset(t12, 0.0)
        st.t12 = t12
        for c in range(NCH):
            z1 = ps_z1.tile([64, CH], F32, tag="z1")
            nc.tensor.matmul(z1, w1b[0], st.xb[:, c * CH:(c + 1) * CH],
                             start=True, stop=False)
            nc.tensor.matmul(z1, w1b[1], st.xb[:, N + c * CH:N + (c + 1) * CH],
                             start=False, stop=True)
            tv = _padded(t12, 0, 64, c, 0)
            zv = z1.rearrange("p (h w) -> p h w", h=RPC)
            nc.scalar.activation(tv, zv, AF.Relu, bias=st.b1v, scale=st.s1v)
        return st

    # ---------------- bisection (PAIR samples) ----------------
    def bisect(sts, q):
        lo = upool.tile([UP, PAIR], U32, name=f"lo_q{q}", tag="lo")
        nc.vector.memset(lo, 0)
        mt = upool.tile([UP, PAIR], U32, tag="mt")
        csum = upool.tile([UP, PAIR], F32, tag="csum")
        scr = upool.tile([UP, UF], F32, tag="uscr")
        for bit in range(31, -1, -1):
            nc.vector.tensor_tensor(mt, lo, bits[:, bit:bit + 1].broadcast_to([UP, PAIR]),
                                    op=OP.bitwise_or)
            for i, st in enumerate(sts):
                nc.vector.scalar_tensor_tensor(
                    scr, st.u, 0, mt[:, i:i + 1].broadcast_to([UP, UF]),
                    op0=OP.bypass, op1=OP.is_gt, accum_out=csum[:, i:i + 1])
            cps = ps_sm.tile([128, PAIR], F32, tag="sm")
            nc.tensor.matmul(cps, onesum, csum, start=True, stop=True)
            flag = upool.tile([UP, PAIR], U32, tag="flag")
            nc.vector.tensor_scalar(flag, cps[0:UP], float(K_SP), float(2 ** bit),
                                    op0=OP.is_ge, op1=OP.mult)
            nc.vector.tensor_tensor(lo, lo, flag, op=OP.bitwise_or)
        for i, st in enumerate(sts):
            st.lo, st.lo_i = lo, i

    # ---------------- stage C (per sample) ----------------
    def stage_c(s, st):
        # spatial mask in u-layout, bf16 (0/1 exact)
        mtile = upool.tile([UP, UF], BF16, tag="mask")
        nc.vector.tensor_tensor(mtile, st.u,
                                st.lo[:, st.lo_i:st.lo_i + 1].broadcast_to([UP, UF]),
                                op=OP.is_gt)
        if DEBUG:
            mdbg = upool.tile([UP, UF], F32, tag="maskdbg", bufs=1)
            nc.vector.tensor_copy(mdbg, mtile)
            nc.sync.dma_start(dbg["mask"][s], mdbg)

        # rhs65 padded layout [65, NP]: rows 0:64 masked conv2 out, row 64 mask
        rhs65 = rhs65p.tile([65, NP], BF16, tag="rhs65")
        if s < 2:
            nc.gpsimd.memset(rhs65, 0.0)
        # mask -> padded row 64: flat pixel 28p+f at BASE + 58h + 28q + f.
        # Two 3-dim DMAs (even/odd partitions) since DMA balancing caps at 3 dims.
        for par in range(2):
            msrc = bass.AP(mtile.tensor, par * UF, [[2 * UF, H], [1, UF]])
            mdst = bass.AP(rhs65.tensor, 64 * NP + BASE + par * UF,
                           [[NP, 1], [PW, H], [1, UF]])
            nc.sync.dma_start(mdst, msrc)
        # 9 shifted copies of the mask row in one 9-descriptor DMA:
        # sh9[j, i] = maskrow[i + 58*(j//3 - 1) + (j%3 - 1)]
        sh9 = sh9p.tile([9, NP], BF16, tag="sh9")
        L = NP - 2 * PW - 2
        for dy in range(3):
            sdy = bass.AP(rhs65.tensor, 64 * NP + dy * PW, [[NP, 1], [1, 3], [1, L]])
            nc.sync.dma_start(sh9[3 * dy:3 * dy + 3, PW + 1:PW + 1 + L], sdy)

        t12 = st.t12
        # dilation: cnt9 matmul + dil = min(cnt,1) multiplied into t12
        for c in range(NCH):
            cnt9 = ps_dm.tile([64, CH], F32, tag="dm")
            nc.tensor.matmul(cnt9, ones9b, _padded(sh9, 0, 9, c, 0),
                             start=True, stop=True)
            tv = _padded(t12, 0, 64, c, 0)
            cv = cnt9.rearrange("p (h w) -> p h w", h=RPC)
            nc.vector.scalar_tensor_tensor(tv, cv, 1.0, tv, op0=OP.min, op1=OP.mult)
        # pre-shifted copy (+1 col) for the dx=(-1,0) K=128 tap pairs
        nc.gpsimd.dma_start(t12[64:128, 0:NP - 1], t12[0:64, 1:NP])
        if DEBUG:
            t12f = upool.tile([128, NP], F32, tag="t12dbg", bufs=1)
            nc.vector.tensor_copy(t12f, t12)
            nc.sync.dma_start(dbg["t12"][s], t12f)

        for c in range(NCH):
            z2 = ps_z2.tile([64, CH], F32, tag="z2")
            for i, dy in enumerate((-1, 0, 1)):
                nc.tensor.matmul(z2, w2pair[i],
                                 _padded(t12, 0, 128, c, PW * dy - 1),
                                 start=(i == 0), stop=False)
                nc.tensor.matmul(z2, w2sing[i],
                                 _padded(t12, 0, 64, c, PW * dy + 1),
                                 start=False, stop=(i == 2))
            r2 = outp.tile([64, CH], BF16, tag="r2")
            nc.scalar.activation(r2, z2, AF.Relu, bias=st.b2v, scale=st.s2v)
            # mask broadcast (K=1 matmul from the rhs65 mask row) and multiply
            mbc = ps_dm.tile([64, CH], F32, tag="dm")
            nc.tensor.matmul(mbc, ones_at64[64:65], _padded(rhs65, 64, 65, c, 0),
                             start=True, stop=True)
            rv = _padded(rhs65, 0, 64, c, 0)
            mv = mbc.rearrange("p (h w) -> p h w", h=RPC)
            r2v = r2.rearrange("p (h w) -> p h w", h=RPC)
            nc.vector.scalar_tensor_tensor(rv, mv, 1.0, r2v, op0=OP.bypass, op1=OP.mult)

        ybuf = []
        for m in range(2):
            yb = ybufp.tile([128, N], BF16, name=f"y{m}_s{s}", tag="ybuf")
            ybuf.append(yb)
        for c in range(NCH):
            for m in range(2):
                z3 = ps_z3.tile([128, CH], F32, tag="z3")
                nc.tensor.matmul(z3, w3[:, 128 * m:128 * (m + 1)],
                                 _padded(rhs65, 0, 65, c, 0), start=True, stop=False)
                nc.tensor.matmul(z3, eye_b,
                                 st.xb[:, m * N + c * CH:m * N + (c + 1) * CH],
                                 start=False, stop=True)
                nc.scalar.activation(ybuf[m][:, c * CH:(c + 1) * CH], z3, AF.Relu)
        for m in range(2):
            nc.gpsimd.dma_start(y_d[s, 128 * m:128 * (m + 1)], ybuf[m])

    # ---------------- schedule: pair-pipelined ----------------
    prev = None
    for q in range(SPC // PAIR):
        sts = [stage_a(q * PAIR + i) for i in range(PAIR)]
        bisect(sts, q)
        if prev is not None:
            for i, st in enumerate(prev):
                stage_c((q - 1) * PAIR + i, st)
        prev = sts
    for i, st in enumerate(prev):
        stage_c((SPC // PAIR - 1) * PAIR + i, st)


_CACHED = {}
LAST_RESULTS = None


def _get_nc():
    if "nc" not in _CACHED:
        _CACHED["nc"] = _build_nc()
    return _CACHED["nc"]


def kernel(**inputs):
    from concourse.bass_utils import run_bass_kernel_spmd
    nc = _get_nc()
    x = np.ascontiguousarray(np.asarray(inputs["x"], np.float32).reshape(B, CIN, N))
    base = {
        "conv1_w": np.ascontiguousarray(np.asarray(inputs["conv1_w"], np.float32).reshape(WIDTH, CIN)),
        "conv2_w": np.ascontiguousarray(np.asarray(inputs["conv2_w"], np.float32)),
        "conv3_w": np.ascontiguousarray(np.asarray(inputs["conv3_w"], np.float32).reshape(COUT, WIDTH)),
        "fc_w": np.ascontiguousarray(np.asarray(inputs["fc_w"], np.float32)),
        "fc_b": np.ascontiguousarray(np.asarray(inputs["fc_b"], np.float32)),
        "mask_w": np.ascontiguousarray(np.asarray(inputs["mask_w"], np.float32).reshape(CIN)),
        "mask_b": np.ascontiguousarray(np.asarray(inputs["mask_b"], np.float32)),
    }
    for pre in ("bn1", "bn2", "bn3"):
        for k in "gbmv":
            base[f"{pre}_{k}"] = np.ascontiguousarray(np.asarray(inputs[f"{pre}_{k}"], np.float32))
    in_maps = []
    for c in range(NCORES):
        m = dict(base)
        m["x"] = np.ascontiguousarray(x[c * SPC:(c + 1) * SPC])
        in_maps.append(m)
    res = run_bass_kernel_spmd(nc, in_maps, core_ids=list(range(NCORES)))
    global LAST_RESULTS
    LAST_RESULTS = res
    y = np.concatenate([np.asarray(r["y"]).astype(np.float32) for r in res.results], axis=0)
    return y.reshape(B, COUT, H, W)
